# revision 1
# baseline (speedup 1.0000x reference)
"""GQA attention kernel for Trainium2, tensor-parallel over heads across 8 NeuronCores.

Problem: x[1,2048,4096] @ {wq[4096,4096], wk/wv[4096,1024]} -> RoPE -> causal GQA
(32 q heads, 8 kv groups, hd=128) -> @ wo[4096,4096].

Sharding: core c owns query heads 4c..4c+3 and KV group c (column shards of
wq/wk/wv).  Context (ctx^T) is AllGathered (4MB/core) and the output projection
is column-sharded (wo columns 512c..512c+512), so no AllReduce is needed.
Each core returns out^T[512, 2048]; the host transposes and concatenates.

All matmuls run as float32r (full-rate fp32 storage, ~1.5e-4 rel err).
RoPE interleaved pairs are pre-permuted into rotate-half layout by permuting
wq/wk columns on the host.  Softmax skips max-subtraction (logits are O(10)),
so scores stream chunk-by-chunk through exp with running row sums.
"""
import os
import sys

sys.path.insert(0, "/opt/trn_rl_repo")

import numpy as np

import concourse.bass as bass
import concourse.mybir as mybir
import concourse.tile as tile
from concourse import bacc
from concourse.bass_utils import run_bass_kernel_spmd

F32 = mybir.dt.float32
F32R = mybir.dt.float32r
AF = mybir.ActivationFunctionType

N_CORES = 8
S = 2048          # sequence length
D = 4096          # model dim
HD = 128          # head dim
NH_PER = 4        # query heads per core
ROPE_BASE = 10000.0
SCALE = 1.0 / float(np.sqrt(HD))
NEG = -1.0e30

ST = S // 128     # 16 sequence tiles of 128
KC = D // 128     # 32 feature chunks of 128
NB = S // 512     # 4 blocks of 512

DEBUG = bool(int(os.environ.get("KERNEL_DEBUG", "0")))
# 1 = QKV+RoPE only, 2 = +attention, 3 = full (AG + out-proj)
PHASES = int(os.environ.get("KERNEL_PHASES", "3"))
NO_AG = bool(int(os.environ.get("KERNEL_NO_AG", "0")))

_NC_CACHE = {}


def build_nc():
    nc = bacc.Bacc("TRN2", target_bir_lowering=False, debug=False,
                   num_devices=N_CORES)

    xt_d = nc.dram_tensor("xt", [ST, 128, D], F32R, kind="ExternalInput")
    wq_d = nc.dram_tensor("wq", [KC, 128, NH_PER * HD], F32R, kind="ExternalInput")
    wkv_d = nc.dram_tensor("wkv", [KC, 128, 2 * HD], F32R, kind="ExternalInput")
    wo_d = nc.dram_tensor("wo", [KC, 128, NH_PER * HD], F32R, kind="ExternalInput")
    sin_d = nc.dram_tensor("sin_t", [128, S], F32R, kind="ExternalInput")
    cos_d = nc.dram_tensor("cos_t", [128, S], F32R, kind="ExternalInput")
    mask_d = nc.dram_tensor("mask_t", [128, 2048], F32, kind="ExternalInput")
    ident_d = nc.dram_tensor("ident", [128, 128], F32R, kind="ExternalInput")

    outT_d = nc.dram_tensor("outT", [512, S], F32, kind="ExternalOutput")

    ctxl_d = nc.dram_tensor("ctxl", [NH_PER * HD, S], F32)
    ctxf_d = nc.dram_tensor("ctxf", [N_CORES * NH_PER * HD, S], F32,
                            addr_space="Shared")
    if DEBUG:
        qt_dbg = nc.dram_tensor("qt_dbg", [128, S], F32, kind="ExternalOutput")
        kt_dbg = nc.dram_tensor("kt_dbg", [128, S], F32, kind="ExternalOutput")
        v_dbg = nc.dram_tensor("v_dbg", [128, S], F32, kind="ExternalOutput")
        ctx_dbg = nc.dram_tensor("ctx_dbg", [NH_PER * HD, S], F32,
                                 kind="ExternalOutput")

    with tile.TileContext(nc) as tc:
        with tc.tile_pool(name="per", bufs=1) as per:
            ident_sb = per.tile([128, 128], F32R, tag="ident")
            nc.sync.dma_start(ident_sb[:], ident_d[:])

            with tc.tile_pool(name="qkvp", bufs=1) as qkvp:
                qt_sb = [qkvp.tile([128, S], F32R, tag=f"qt{h}", name=f"qt{h}")
                         for h in range(NH_PER)]
                kt_sb = qkvp.tile([128, S], F32R, tag="kt")
                v_sb = qkvp.tile([128, S], F32R, tag="v")

                # ---------------- Phase 1: QKV projections ----------------
                with tc.tile_pool(name="w1", bufs=1) as w1, \
                     tc.tile_pool(name="xp", bufs=2) as xp, \
                     tc.tile_pool(name="stq", bufs=3) as stq, \
                     tc.tile_pool(name="ps1", bufs=2, space="PSUM") as ps1:
                    wq_sb = w1.tile([128, KC * NH_PER * HD], F32R, tag="wq")
                    wkv_sb = w1.tile([128, KC * 2 * HD], F32R, tag="wkv")
                    nc.sync.dma_start(
                        wq_sb[:].rearrange("p (kc c) -> p kc c", kc=KC),
                        wq_d[:].rearrange("kc p c -> p kc c"))
                    nc.sync.dma_start(
                        wkv_sb[:].rearrange("p (kc c) -> p kc c", kc=KC),
                        wkv_d[:].rearrange("kc p c -> p kc c"))

                    for st in range(ST):
                        xa = xp.tile([128, 16 * 128], F32R, tag="x", name="xa")
                        xb = xp.tile([128, 16 * 128], F32R, tag="x", name="xb")
                        nc.sync.dma_start(xa[:], xt_d[st, :, 0:2048])
                        nc.sync.dma_start(xb[:], xt_d[st, :, 2048:4096])
                        q_ps = ps1.tile([128, NH_PER * HD], F32, tag="q")
                        kv_ps = ps1.tile([128, 2 * HD], F32, tag="kv")
                        for kc in range(KC):
                            xs = (xa if kc < 16 else xb)[
                                :, (kc % 16) * 128:(kc % 16 + 1) * 128]
                            nc.tensor.matmul(q_ps[:], xs,
                                             wq_sb[:, kc * 512:(kc + 1) * 512],
                                             start=(kc == 0), stop=(kc == KC - 1))
                            nc.tensor.matmul(kv_ps[:], xs,
                                             wkv_sb[:, kc * 256:(kc + 1) * 256],
                                             start=(kc == 0), stop=(kc == KC - 1))
                        qstage = stq.tile([128, NH_PER * HD], F32R, tag="qst")
                        kvstage = stq.tile([128, 2 * HD], F32R, tag="kvst")
                        nc.scalar.copy(qstage[:], q_ps[:])
                        nc.vector.tensor_copy(kvstage[:], kv_ps[:])
                        cs = slice(st * 128, (st + 1) * 128)
                        for h in range(NH_PER):
                            tr = ps1.tile([128, 128], F32R, tag="tr", name="tr")
                            nc.tensor.transpose(tr[:],
                                                qstage[:, h * 128:(h + 1) * 128],
                                                ident_sb[:])
                            nc.vector.tensor_copy(qt_sb[h][:, cs], tr[:])
                        trk = ps1.tile([128, 128], F32R, tag="tr")
                        nc.tensor.transpose(trk[:], kvstage[:, 0:128], ident_sb[:])
                        nc.vector.tensor_copy(kt_sb[:, cs], trk[:])
                        nc.scalar.copy(v_sb[:, cs], kvstage[:, 128:256])

                # ---------------- Phase 1.5: RoPE on qT, kT ----------------
                # tables duplicated on both partition halves (DVE needs equal
                # input base partitions)
                with tc.tile_pool(name="rp", bufs=2) as rp:
                    sin_sb = rp.tile([128, S], F32R, tag="sin", bufs=1)
                    cos_sb = rp.tile([128, S], F32R, tag="cos", bufs=1)
                    nc.sync.dma_start(sin_sb[:], sin_d[:])
                    nc.sync.dma_start(cos_sb[:], cos_d[:])
                    for T in qt_sb + [kt_sb]:
                        for ch in range(2):
                            cs = slice(ch * 1024, (ch + 1) * 1024)
                            lo = T[0:64, cs]
                            hi = T[64:128, cs]
                            slo = sin_sb[0:64, cs]
                            shi = sin_sb[64:128, cs]
                            clo = cos_sb[0:64, cs]
                            chi = cos_sb[64:128, cs]
                            t1 = rp.tile([64, 1024], F32R, tag="rt1")
                            t2 = rp.tile([64, 1024], F32R, tag="rt2")
                            t3 = rp.tile([64, 1024], F32R, tag="rt3")
                            t4 = rp.tile([64, 1024], F32R, tag="rt4")
                            nc.vector.tensor_mul(t1[:], lo, slo)
                            nc.vector.tensor_mul(t2[:], lo, clo)
                            nc.vector.tensor_mul(t3[:], hi, shi)
                            nc.vector.tensor_sub(lo, t2[:], t3[:])
                            nc.vector.tensor_mul(t4[:], hi, chi)
                            nc.vector.tensor_add(hi, t4[:], t1[:])

                if DEBUG:
                    with tc.tile_pool(name="dbg", bufs=1) as dbs:
                        d1 = dbs.tile([128, S], F32, tag="d1")
                        nc.vector.tensor_copy(d1[:], qt_sb[0][:].bitcast(F32))
                        nc.sync.dma_start(qt_dbg[:], d1[:])
                        d2 = dbs.tile([128, S], F32, tag="d2")
                        nc.vector.tensor_copy(d2[:], kt_sb[:].bitcast(F32))
                        nc.sync.dma_start(kt_dbg[:], d2[:])
                        d3 = dbs.tile([128, S], F32, tag="d3")
                        nc.vector.tensor_copy(d3[:], v_sb[:].bitcast(F32))
                        nc.sync.dma_start(v_dbg[:], d3[:])

                # ---------------- Phase 2: attention per head ----------------
                if PHASES < 2:
                    nc.compile()
                    return nc
                with tc.tile_pool(name="pp", bufs=5) as pp, \
                     tc.tile_pool(name="pts", bufs=3) as pts, \
                     tc.tile_pool(name="m2", bufs=4) as m2, \
                     tc.tile_pool(name="ps2", bufs=2, space="PSUM") as ps2:
                    mask_sb = m2.tile([128, 2048], F32, tag="mask", bufs=1)
                    nc.sync.dma_start(mask_sb[:], mask_d[:])
                    for h in range(NH_PER):
                        for B in range(NB):
                            nch = B + 1  # number of 512-wide k chunks
                            p_list = []
                            for tl in range(4):
                                tg = 4 * B + tl
                                p_t = pp.tile([128, 2048], F32R, tag="p",
                                              name=f"p{tl}")
                                chs = m2.tile([128, 4], F32, tag="chs")
                                for c in range(nch):
                                    s_ps = ps2.tile([128, 512], F32, tag="s")
                                    nc.tensor.matmul(
                                        s_ps[:],
                                        qt_sb[h][:, tg * 128:(tg + 1) * 128],
                                        kt_sb[:, c * 512:(c + 1) * 512],
                                        start=True, stop=True)
                                    if c == B:
                                        nc.vector.tensor_add(
                                            s_ps[:], s_ps[:],
                                            mask_sb[:, tl * 512:(tl + 1) * 512])
                                    nc.scalar.activation(
                                        p_t[:, c * 512:(c + 1) * 512], s_ps[:],
                                        AF.Exp, bias=0.0, scale=SCALE,
                                        accum_out=chs[:, c:c + 1])
                                rs = m2.tile([128, 1], F32, tag="rs")
                                rinv = m2.tile([128, 1], F32, tag="rinv")
                                nc.vector.reduce_sum(rs[:], chs[:, 0:nch],
                                                     axis=mybir.AxisListType.X)
                                nc.vector.reciprocal(rinv[:], rs[:])
                                nc.vector.tensor_scalar_mul(
                                    p_t[:, 0:nch * 512], p_t[:, 0:nch * 512],
                                    rinv[:])
                                p_list.append(p_t)
                            ctx_ps = ps2.tile([128, 512], F32, tag="ctx")
                            nj = 4 * nch
                            for j in range(nj):
                                pt4 = pts.tile([128, 512], F32R, tag="pt4")
                                for tl in range(4):
                                    trp = ps2.tile([128, 128], F32R, tag="ptr",
                                                   name="trp")
                                    nc.tensor.transpose(
                                        trp[:],
                                        p_list[tl][:, j * 128:(j + 1) * 128],
                                        ident_sb[:])
                                    nc.vector.tensor_copy(
                                        pt4[:, tl * 128:(tl + 1) * 128], trp[:])
                                nc.tensor.matmul(
                                    ctx_ps[:], v_sb[:, j * 128:(j + 1) * 128],
                                    pt4[:], start=(j == 0), stop=(j == nj - 1))
                            cstage = m2.tile([128, 512], F32, tag="cst")
                            nc.scalar.copy(cstage[:], ctx_ps[:])
                            nc.sync.dma_start(
                                ctxl_d[h * 128:(h + 1) * 128,
                                       B * 512:(B + 1) * 512], cstage[:])

            # ---------------- Phase 2.9: AllGather ctx^T ----------------
            nc.gpsimd.collective_compute(
                "AllGather", mybir.AluOpType.bypass,
                ins=[ctxl_d[:]], outs=[ctxf_d[:]],
                replica_groups=[list(range(N_CORES))])
            if DEBUG:
                with tc.tile_pool(name="dbg2", bufs=1) as db2:
                    dc = db2.tile([128, NH_PER * S], F32, tag="dc")
                    nc.sync.dma_start(
                        dc[:].rearrange("p (n s) -> p n s", n=NH_PER),
                        ctxl_d[:].rearrange("(n p) s -> p n s", p=128))
                    nc.sync.dma_start(
                        ctx_dbg[:].rearrange("(n p) s -> p n s", p=128),
                        dc[:].rearrange("p (n s) -> p n s", n=NH_PER))

            # ---------------- Phase 3: output projection ----------------
            with tc.tile_pool(name="cq", bufs=1) as cqp, \
                 tc.tile_pool(name="wop", bufs=6) as wop, \
                 tc.tile_pool(name="m3", bufs=4) as m3, \
                 tc.tile_pool(name="ps3", bufs=4, space="PSUM") as ps3:
                for half in range(2):
                    hs = slice(half * 1024, (half + 1) * 1024)
                    cq = cqp.tile([128, KC * 1024], F32R, tag="cq")
                    nc.sync.dma_start(
                        cq[:].rearrange("p (kc s) -> p kc s", kc=KC),
                        ctxf_d[:].rearrange("(kc p) s -> p kc s", p=128)
                        .bitcast(F32R)[:, :, hs])
                    for oc in range(4):
                        o_ps = [ps3.tile([128, 512], F32, tag="o", name=f"o{i}")
                                for i in range(2)]
                        for kc in range(KC):
                            wot = wop.tile([128, 128], F32R, tag="wot")
                            nc.sync.dma_start(
                                wot[:], wo_d[kc, :, oc * 128:(oc + 1) * 128])
                            for sb in range(2):
                                nc.tensor.matmul(
                                    o_ps[sb][:], wot[:],
                                    cq[:, kc * 1024 + sb * 512:
                                       kc * 1024 + (sb + 1) * 512],
                                    start=(kc == 0), stop=(kc == KC - 1))
                        for sb in range(2):
                            ost = m3.tile([128, 512], F32, tag="ost")
                            nc.scalar.copy(ost[:], o_ps[sb][:])
                            nc.sync.dma_start(
                                outT_d[oc * 128:(oc + 1) * 128,
                                       half * 1024 + sb * 512:
                                       half * 1024 + (sb + 1) * 512],
                                ost[:])
    nc.compile()
    return nc


def _host_prep(x, wq, wk, wv, wo):
    """Builds per-core input maps (all host-side numpy, cheap)."""
    x2 = np.ascontiguousarray(x.reshape(S, D), dtype=np.float32)
    xt_in = np.ascontiguousarray(
        x2.reshape(ST, 128, KC, 128).transpose(0, 3, 2, 1).reshape(ST, 128, D))

    perm = np.concatenate([np.arange(0, HD, 2), np.arange(1, HD, 2)])
    half = HD // 2
    inv = ROPE_BASE ** (-np.arange(half, dtype=np.float64) / half)
    ang = np.arange(S, dtype=np.float64)[None, :] * inv[:, None]
    sin_t = np.sin(ang).astype(np.float32)
    cos_t = np.cos(ang).astype(np.float32)
    sin_t = np.ascontiguousarray(np.concatenate([sin_t, sin_t], axis=0))
    cos_t = np.ascontiguousarray(np.concatenate([cos_t, cos_t], axis=0))

    mask_t = np.zeros((128, 2048), dtype=np.float32)
    ii = np.arange(128)[:, None]
    jj = np.arange(512)[None, :]
    for tl in range(4):
        mask_t[:, tl * 512:(tl + 1) * 512] = np.where(jj <= tl * 128 + ii, 0.0, NEG)
    ident = np.eye(128, dtype=np.float32)

    in_maps = []
    for c in range(N_CORES):
        wqc = wq[:, c * 512:(c + 1) * 512].reshape(D, NH_PER, HD)[:, :, perm]
        wqc = np.ascontiguousarray(wqc.reshape(D, 512).reshape(KC, 128, 512))
        wkc = wk[:, c * HD:(c + 1) * HD][:, perm]
        wvc = wv[:, c * HD:(c + 1) * HD]
        wkvc = np.ascontiguousarray(
            np.concatenate([wkc, wvc], axis=1).reshape(KC, 128, 2 * HD))
        woc = np.ascontiguousarray(
            wo[:, c * 512:(c + 1) * 512].reshape(KC, 128, 512))
        in_maps.append({
            "xt": xt_in, "wq": wqc, "wkv": wkvc, "wo": woc,
            "sin_t": sin_t, "cos_t": cos_t, "mask_t": mask_t, "ident": ident,
        })
    return in_maps


def kernel(x, wq, wk, wv, wo):
    if "nc" not in _NC_CACHE:
        _NC_CACHE["nc"] = build_nc()
    nc = _NC_CACHE["nc"]
    in_maps = _host_prep(np.asarray(x), np.asarray(wq), np.asarray(wk),
                         np.asarray(wv), np.asarray(wo))
    res = run_bass_kernel_spmd(nc, in_maps, core_ids=list(range(N_CORES)))
    _NC_CACHE["last_results"] = res
    out = np.empty((S, D), dtype=np.float32)
    for c in range(N_CORES):
        out[:, c * 512:(c + 1) * 512] = res.results[c]["outT"].T
    return out.reshape(1, S, D)

